# revision 18
# baseline (speedup 1.0000x reference)
"""Trainium2 Bass kernel for nn_Decoder (2-layer LSTM decoder with output feedback).

Sharding: 8-way tensor parallel over the hidden dimension. Core d owns hidden
units [d*256, (d+1)*256) of each layer: the 4*256=1024 gate rows of W_ih/W_hh
(rows g*H + d*256 .. for each gate g), and the matching slices of c and h.
All weights stay SBUF-resident in bf16 for all 25 timesteps. h slices are
exchanged with an AllGather per layer per step, in transposed [256, 64] layout
so the gathered buffer [2048, 64] is directly the stationary matmul operand
layout for the next matmuls. Cell state and gate accumulation stay fp32.
"""
import sys
import numpy as np

for _p in ("/opt/trn_rl_repo", "/root/.axon_site/_ro/trn_rl_repo"):
    if _p not in sys.path:
        sys.path.append(_p)

import ml_dtypes

B, T, NL, H, IN, OUT = 64, 25, 2, 2048, 66, 66
R = 8            # cores
HS = H // R      # 256 hidden units per core
G = 4 * HS       # 1024 gate rows per core
NK = H // 128    # 16 contraction chunks
BF = ml_dtypes.bfloat16

_CACHE = {}


def _build():
    from concourse import bacc, tile, mybir

    bf16 = mybir.dt.bfloat16
    f32 = mybir.dt.float32

    nc = bacc.Bacc("TRN2", target_bir_lowering=False, debug=False, num_devices=R)

    # --- DRAM I/O (per-core shards prepared on host) ---
    d_wih0 = nc.dram_tensor("wih0", [IN + 1, G], bf16, kind="ExternalInput")   # W_ih0 sliceT + bias row
    d_whh0 = nc.dram_tensor("whh0", [H, G], bf16, kind="ExternalInput")
    d_wih1 = nc.dram_tensor("wih1", [H, G], bf16, kind="ExternalInput")
    d_whh1 = nc.dram_tensor("whh1", [H, G], bf16, kind="ExternalInput")
    d_b1 = nc.dram_tensor("b1", [1, G], bf16, kind="ExternalInput")
    d_wf = nc.dram_tensor("wf", [H, G], bf16, kind="ExternalInput")            # (W_ih0 @ fc_w).T slice
    d_b0p = nc.dram_tensor("b0p", [1, G], bf16, kind="ExternalInput")          # b0 + W_ih0 @ fc_b
    d_fcw = nc.dram_tensor("fcw", [H, OUT], bf16, kind="ExternalInput")        # fc_w.T
    d_fcb = nc.dram_tensor("fcb", [1, OUT], bf16, kind="ExternalInput")
    d_x0 = nc.dram_tensor("x0", [IN + 1, B], bf16, kind="ExternalInput")       # x0.T + ones row
    d_h0 = nc.dram_tensor("h0", [H, B], bf16, kind="ExternalInput")            # hiddens[0].T
    d_h1 = nc.dram_tensor("h1", [H, B], bf16, kind="ExternalInput")
    d_c0 = nc.dram_tensor("c0", [B, HS], f32, kind="ExternalInput")
    d_c1 = nc.dram_tensor("c1", [B, HS], f32, kind="ExternalInput")
    d_ones = nc.dram_tensor("ones", [1, B], bf16, kind="ExternalInput")
    d_id = nc.dram_tensor("id64", [B, B], f32, kind="ExternalInput")
    d_out = nc.dram_tensor("out", [T, OUT, B], f32, kind="ExternalOutput")

    AF = mybir.ActivationFunctionType
    ALU = mybir.AluOpType
    groups = [list(range(R))]

    with tile.TileContext(nc) as tc:
        with (
            tc.tile_pool(name="wpool", bufs=1) as wpool,
            tc.tile_pool(name="state", bufs=1) as state,
            tc.tile_pool(name="work", bufs=2) as work,
            tc.tile_pool(name="psA", bufs=2, space="PSUM") as psA,
            tc.tile_pool(name="psT", bufs=2, space="PSUM") as psTpool,
            tc.tile_pool(name="psF", bufs=1, space="PSUM") as psF,
            tc.tile_pool(name="psH", bufs=1, space="PSUM") as psH,
            tc.tile_pool(name="dram", bufs=2, space="DRAM") as dram,
        ):
            # --- persistent SBUF tiles ---
            wih0 = wpool.tile([IN + 1, G], bf16, tag="wih0")
            whh0 = wpool.tile([128, NK * G], bf16, tag="whh0")
            wih1 = wpool.tile([128, NK * G], bf16, tag="wih1")
            whh1 = wpool.tile([128, NK * G], bf16, tag="whh1")
            b1 = wpool.tile([1, G], bf16, tag="b1")
            wf = wpool.tile([128, NK * G], bf16, tag="wf")
            b0p = wpool.tile([1, G], bf16, tag="b0p")
            fcw = wpool.tile([128, NK * OUT], bf16, tag="fcw")
            fcb = wpool.tile([1, OUT], bf16, tag="fcb")
            ones = wpool.tile([1, B], bf16, tag="ones")
            id64 = wpool.tile([B, B], f32, tag="id64")

            xT = state.tile([IN + 1, B], bf16, tag="xT")       # x.T (+ones row 66)
            h0T = state.tile([128, NK * B], bf16, tag="h0T")   # h0.T chunks
            h1T = state.tile([128, NK * B], bf16, tag="h1T")
            c0 = state.tile([B, HS], f32, tag="c0")
            c1 = state.tile([B, HS], f32, tag="c1")

            # --- initial loads (batched 3D-AP DMAs, split across both HWDGE
            # sequencers; h/x/c first so step 0 can start) ---
            def load_T(sb, dr, n, eng, parts=2):
                nk = dr.shape[0] // 128
                for i in range(parts):
                    a, b = i * nk // parts, (i + 1) * nk // parts
                    eng.dma_start(
                        sb[:, a * n:b * n].rearrange("p (k f) -> p k f", f=n),
                        dr[a * 128:b * 128, :].rearrange("(k p) f -> p k f", p=128))
            nc.sync.dma_start(xT[:], d_x0[:])
            nc.sync.dma_start(c0[:], d_c0[:])
            nc.sync.dma_start(c1[:], d_c1[:])
            load_T(h0T, d_h0, B, nc.scalar, 1)
            load_T(h1T, d_h1, B, nc.sync, 1)
            nc.scalar.dma_start(wih0[:], d_wih0[:])
            load_T(whh0, d_whh0, G, nc.scalar)
            load_T(whh1, d_whh1, G, nc.sync)
            load_T(wih1, d_wih1, G, nc.scalar)
            load_T(wf, d_wf, G, nc.sync)
            nc.sync.dma_start(b0p[:], d_b0p[:])
            load_T(fcw, d_fcw, OUT, nc.sync, 1)
            nc.sync.dma_start(b1[:], d_b1[:])
            nc.sync.dma_start(fcb[:], d_fcb[:])
            nc.sync.dma_start(ones[:], d_ones[:])
            nc.sync.dma_start(id64[:], d_id[:])

            def heater(n):
                """Dummy K=1 matmuls that keep the PE HAM clock at 2.4GHz
                through a collective window (idle >3.4us re-throttles to
                1.2GHz and the next real burst would run ~2x slow)."""
                hp = psH.tile([B, 512], f32, tag="heat")
                for i in range(n):
                    nc.tensor.matmul(hp[:], ones[0:1, :], b1[0:1, 0:512],
                                     start=(i == 0), stop=(i == n - 1))

            def fc_block(t):
                """out[t] = clip(fc(h1(t))); writes out DRAM row t and xT."""
                ps = psF.tile([OUT, B], f32, tag="fc")
                for k in range(NK):
                    nc.tensor.matmul(
                        ps[:], fcw[:, k * OUT:(k + 1) * OUT], h1T[:, k * B:(k + 1) * B],
                        start=(k == 0), stop=False)
                nc.tensor.matmul(ps[:], fcb[0:1, :], ones[0:1, :], start=False, stop=True)
                o_sb = work.tile([OUT, B], f32, tag="osb")
                nc.vector.tensor_scalar(o_sb[:], ps[:], 1.0, -1.0, op0=ALU.min, op1=ALU.max)
                nc.sync.dma_start(d_out[t], o_sb[:])

            def gates_nonlin(layer, ps, c, t):
                """PSUM gates [128, 512] -> h sliceT bf16 in DRAM bounce.

                gate row order is [i, f, o, g] (host reorders); col-group A
                wrote gate cols 0:512 (i, f) on partitions 0:64, col-group B
                wrote cols 512:1024 (o, g) on partitions 64:128."""
                act = work.tile([B, G], f32, tag="gact")
                nc.scalar.activation(act[:, 0:512], ps[0:B, :], AF.Sigmoid)              # i, f
                nc.scalar.activation(act[:, 3 * HS:4 * HS], ps[B:2 * B, HS:2 * HS], AF.Tanh)  # g
                nc.scalar.activation(act[:, 512:3 * HS], ps[B:2 * B, 0:HS], AF.Sigmoid)     # o
                # state update pipelined in two hidden-unit halves so the
                # transpose/cast/DMA of half 0 overlaps the math of half 1
                ag_in = dram.tile([HS, B], bf16, tag=f"agin{layer}")
                pst = psTpool.tile([128, 128], f32, tag="tr")
                hTbf = work.tile([128, 2 * B], bf16, tag="hTbf")
                t1 = work.tile([B, HS], f32, tag="t1")
                t2 = work.tile([B, HS], f32, tag="t2")
                tc_ = work.tile([B, HS], f32, tag="tch")
                hs = work.tile([B, HS], f32, tag="hs")
                for u in (0, 1):
                    s = slice(u * 128, (u + 1) * 128)
                    nc.vector.tensor_mul(t2[:, s], act[:, u * 128:u * 128 + 128],
                                         act[:, 3 * HS + u * 128:3 * HS + u * 128 + 128])  # i*g
                    nc.vector.tensor_mul(t1[:, s], act[:, HS + u * 128:HS + u * 128 + 128], c[:, s])  # f*c
                    nc.vector.tensor_add(c[:, s], t1[:, s], t2[:, s])                # c_new
                    nc.scalar.activation(tc_[:, s], c[:, s], AF.Tanh)
                    nc.vector.tensor_mul(hs[:, s], act[:, 2 * HS + u * 128:2 * HS + u * 128 + 128],
                                         tc_[:, s])                                  # o * tanh(c)
                    nc.tensor.transpose(pst[:, u * B:(u + 1) * B], hs[:, s], id64[:])
                    nc.vector.tensor_copy(hTbf[:, u * B:(u + 1) * B], pst[:, u * B:(u + 1) * B])
                    eng = nc.scalar if u == 0 else nc.sync
                    eng.dma_start(ag_in[u * 128:(u + 1) * 128, :], hTbf[:, u * B:(u + 1) * B])
                return ag_in

            def all_gather(layer, ag_in, hT):
                ag_out = dram.tile([H, B], bf16, tag=f"agout{layer}", addr_space="Shared")
                nc.gpsimd.collective_compute(
                    "AllGather", ALU.bypass, replica_groups=groups,
                    ins=[ag_in[:].opt()], outs=[ag_out[:].opt()])
                # staggered quarter readbacks on both HWDGE sequencers so the
                # first consuming matmuls start ~2us earlier
                for q, eng in ((0, nc.scalar), (1, nc.sync), (2, nc.scalar), (3, nc.sync)):
                    eng.dma_start(
                        hT[:, q * 4 * B:(q + 1) * 4 * B].rearrange("p (k f) -> p k f", f=B),
                        ag_out[q * 512:(q + 1) * 512, :].rearrange("(k p) f -> p k f", p=128))

            def mm_group(ps, hT_src, w, start, stop):
                """Col-tiled matmul pairs: group A (PE cols 0:63) computes
                gate cols 0:512 -> partitions 0:64; group B (cols 64:127)
                computes gate cols 512:1024 -> partitions 64:128. The two
                stream concurrently through the PE array."""
                for k in range(NK):
                    for r in range(2):
                        nc.tensor.matmul(
                            ps[r * B:(r + 1) * B, :],
                            hT_src[:, k * B:(k + 1) * B],
                            w[:, k * G + r * 512: k * G + (r + 1) * 512],
                            start=(start and k == 0),
                            stop=(stop and k == NK - 1),
                            tile_position=(0, r * B))

            def xb_pair(ps, lhsT, rhs2, start, stop):
                """One K-chunk into both col groups (x-part / bias)."""
                for r in range(2):
                    nc.tensor.matmul(
                        ps[r * B:(r + 1) * B, :],
                        lhsT, rhs2[0:lhsT.shape[0], r * 512:(r + 1) * 512],
                        start=start, stop=stop, tile_position=(0, r * B))

            for t in range(T):
                # ---- layer 0 gates: h-part first (ready before FC of t-1) ----
                g0 = psA.tile([2 * B, 512], f32, tag="g")
                mm_group(g0, h0T, whh0, start=True, stop=False)
                if t > 0:
                    heater(16)
                if t > 0:
                    # x-feedback fused: W_ih0 @ fc(h1(t-1)) == (W_ih0@fc_w) @ h1(t-1)
                    # (hardtanh never clips at this scale); pipelines with the
                    # AG1(t-1) readback chunk by chunk
                    mm_group(g0, h1T, wf, start=False, stop=False)
                    xb_pair(g0, ones[0:1, :], b0p[:], start=False, stop=True)
                else:
                    xb_pair(g0, xT[:], wih0[:], start=False, stop=True)
                ag0_in = gates_nonlin(0, g0, c0, t)
                # ---- layer 1 h-part (independent of AG0 -> overlaps collective) ----
                g1 = psA.tile([2 * B, 512], f32, tag="g")
                mm_group(g1, h1T, whh1, start=True, stop=False)
                xb_pair(g1, ones[0:1, :], b1[:], start=False, stop=False)
                all_gather(0, ag0_in, h0T)
                # FC output of t-1 (out tensor only, off the feedback path) --
                # PE reaches it during the AG0 mesh; emitted after the
                # collective so the doorbell doesn't pick up its deps
                if t > 0:
                    fc_block(t - 1)
                heater(18)
                # ---- layer 1 x-part: W_ih1 @ h0(t) ----
                mm_group(g1, h0T, wih1, start=False, stop=True)
                ag1_in = gates_nonlin(1, g1, c1, t)
                all_gather(1, ag1_in, h1T)
            fc_block(T - 1)

    nc.compile()
    return nc


def _prep_inputs(inputs, hiddens, cells, W_ih0, W_hh0, b_ih0, b_hh0,
                 W_ih1, W_hh1, b_ih1, b_hh1, fc_w, fc_b):
    """Host-side sharding: per-core in_maps."""
    b0 = np.asarray(b_ih0, np.float32) + np.asarray(b_hh0, np.float32)
    b1 = np.asarray(b_ih1, np.float32) + np.asarray(b_hh1, np.float32)
    onesB = np.ones((1, B), np.float32)
    x0T = np.concatenate([np.asarray(inputs, np.float32).T, onesB], 0)
    shared = {
        "fcw": np.ascontiguousarray(np.asarray(fc_w, np.float32).T).astype(BF),
        "fcb": np.asarray(fc_b, np.float32)[None, :].astype(BF),
        "x0": x0T.astype(BF),
        "h0": np.ascontiguousarray(np.asarray(hiddens[0], np.float32).T).astype(BF),
        "h1": np.ascontiguousarray(np.asarray(hiddens[1], np.float32).T).astype(BF),
        "ones": onesB.astype(BF),
        "id64": np.eye(B, dtype=np.float32),
    }
    in_maps = []
    for d in range(R):
        def rows(W):
            # gate order [i, f, o, g] (torch order is i, f, g, o) — kernel's
            # nonlin slices assume the three sigmoids are contiguous
            W = np.asarray(W, np.float32)
            return np.concatenate([W[g * H + d * HS: g * H + (d + 1) * HS] for g in (0, 1, 3, 2)], 0)
        wih0T = rows(W_ih0).T                      # [66, 1024]
        bias0 = rows(b0[:, None])[:, 0][None, :]   # [1, 1024]
        m = dict(shared)
        m["wih0"] = np.concatenate([wih0T, bias0], 0).astype(BF)
        # fused feedback: W_ih0 @ clip(fc(h1)) == (W_ih0 @ fc_w) @ h1 since the
        # hardtanh never engages at this output scale (|fc| < 0.2)
        wih0_sl = rows(W_ih0)                      # [1024, 66]
        fcw_f = np.asarray(fc_w, np.float32)
        m["wf"] = np.ascontiguousarray((wih0_sl @ fcw_f).T).astype(BF)   # [2048, 1024]
        m["b0p"] = (bias0 + (wih0_sl @ np.asarray(fc_b, np.float32))[None, :]).astype(BF)
        m["whh0"] = np.ascontiguousarray(rows(W_hh0).T).astype(BF)
        m["wih1"] = np.ascontiguousarray(rows(W_ih1).T).astype(BF)
        m["whh1"] = np.ascontiguousarray(rows(W_hh1).T).astype(BF)
        m["b1"] = rows(b1[:, None])[:, 0][None, :].astype(BF)
        m["c0"] = np.ascontiguousarray(np.asarray(cells[0], np.float32)[:, d * HS:(d + 1) * HS])
        m["c1"] = np.ascontiguousarray(np.asarray(cells[1], np.float32)[:, d * HS:(d + 1) * HS])
        in_maps.append(m)
    return in_maps


def _get_nc():
    if "nc" not in _CACHE:
        _CACHE["nc"] = _build()
    return _CACHE["nc"]


def run_raw(in_maps, trace=False, **kw):
    from concourse import bass_utils
    nc = _get_nc()
    return bass_utils.run_bass_kernel_spmd(
        nc, in_maps, core_ids=list(range(R)), trace=trace, **kw)


def kernel(**inputs):
    in_maps = _prep_inputs(**inputs)
    res = run_raw(in_maps)
    outT = np.asarray(res.results[0]["out"])          # [25, 66, 64]
    return np.ascontiguousarray(outT.transpose(2, 0, 1)).astype(np.float32)


# revision 19
# speedup vs baseline: 1.2822x; 1.2822x over previous
"""Trainium2 Bass kernel for nn_Decoder (2-layer LSTM decoder with output feedback).

Sharding: 8-way tensor parallel over the hidden dimension. Core d owns hidden
units [d*256, (d+1)*256) of each layer: the 4*256=1024 gate rows of W_ih/W_hh
(rows g*H + d*256 .. for each gate g), and the matching slices of c and h.
All weights stay SBUF-resident in bf16 for all 25 timesteps. h slices are
exchanged with an AllGather per layer per step, in transposed [256, 64] layout
so the gathered buffer [2048, 64] is directly the stationary matmul operand
layout for the next matmuls. Cell state and gate accumulation stay fp32.
"""
import sys
import numpy as np

for _p in ("/opt/trn_rl_repo", "/root/.axon_site/_ro/trn_rl_repo"):
    if _p not in sys.path:
        sys.path.append(_p)

import ml_dtypes

B, T, NL, H, IN, OUT = 64, 25, 2, 2048, 66, 66
R = 8            # cores
HS = H // R      # 256 hidden units per core
G = 4 * HS       # 1024 gate rows per core
NK = H // 128    # 16 contraction chunks
BF = ml_dtypes.bfloat16

_CACHE = {}


def _build():
    from concourse import bacc, tile, mybir

    bf16 = mybir.dt.bfloat16
    f32 = mybir.dt.float32

    nc = bacc.Bacc("TRN2", target_bir_lowering=False, debug=False, num_devices=R)

    # --- DRAM I/O (per-core shards prepared on host) ---
    d_wih0 = nc.dram_tensor("wih0", [IN + 1, G], bf16, kind="ExternalInput")   # W_ih0 sliceT + bias row
    d_whh0 = nc.dram_tensor("whh0", [H, G], bf16, kind="ExternalInput")
    d_wih1 = nc.dram_tensor("wih1", [H, G], bf16, kind="ExternalInput")
    d_whh1 = nc.dram_tensor("whh1", [H, G], bf16, kind="ExternalInput")
    d_b1 = nc.dram_tensor("b1", [1, G], bf16, kind="ExternalInput")
    d_fcw = nc.dram_tensor("fcw", [H, OUT], bf16, kind="ExternalInput")        # fc_w.T
    d_fcb = nc.dram_tensor("fcb", [1, OUT], bf16, kind="ExternalInput")
    d_x0 = nc.dram_tensor("x0", [IN + 1, B], bf16, kind="ExternalInput")       # x0.T + ones row
    d_h0 = nc.dram_tensor("h0", [H, B], bf16, kind="ExternalInput")            # hiddens[0].T
    d_h1 = nc.dram_tensor("h1", [H, B], bf16, kind="ExternalInput")
    d_c0 = nc.dram_tensor("c0", [B, HS], f32, kind="ExternalInput")
    d_c1 = nc.dram_tensor("c1", [B, HS], f32, kind="ExternalInput")
    d_ones = nc.dram_tensor("ones", [1, B], bf16, kind="ExternalInput")
    d_id = nc.dram_tensor("id64", [B, B], f32, kind="ExternalInput")
    d_out = nc.dram_tensor("out", [T, OUT, B], f32, kind="ExternalOutput")

    AF = mybir.ActivationFunctionType
    ALU = mybir.AluOpType
    groups = [list(range(R))]

    with tile.TileContext(nc) as tc:
        with (
            tc.tile_pool(name="wpool", bufs=1) as wpool,
            tc.tile_pool(name="state", bufs=1) as state,
            tc.tile_pool(name="work", bufs=2) as work,
            tc.tile_pool(name="psA", bufs=2, space="PSUM") as psA,
            tc.tile_pool(name="psT", bufs=2, space="PSUM") as psTpool,
            tc.tile_pool(name="psF", bufs=1, space="PSUM") as psF,
            tc.tile_pool(name="dram", bufs=2, space="DRAM") as dram,
        ):
            # --- persistent SBUF tiles ---
            wih0 = wpool.tile([IN + 1, G], bf16, tag="wih0")
            whh0 = wpool.tile([128, NK * G], bf16, tag="whh0")
            wih1 = wpool.tile([128, NK * G], bf16, tag="wih1")
            whh1 = wpool.tile([128, NK * G], bf16, tag="whh1")
            b1 = wpool.tile([1, G], bf16, tag="b1")
            fcw = wpool.tile([128, NK * OUT], bf16, tag="fcw")
            fcb = wpool.tile([1, OUT], bf16, tag="fcb")
            ones = wpool.tile([1, B], bf16, tag="ones")
            id64 = wpool.tile([B, B], f32, tag="id64")

            xT = state.tile([IN + 1, B], bf16, tag="xT")       # x.T (+ones row 66)
            h0T = state.tile([128, NK * B], bf16, tag="h0T")   # h0.T chunks
            h1T = state.tile([128, NK * B], bf16, tag="h1T")
            c0 = state.tile([B, HS], f32, tag="c0")
            c1 = state.tile([B, HS], f32, tag="c1")

            # --- initial loads (batched 3D-AP DMAs, split across both HWDGE
            # sequencers; h/x/c first so step 0 can start) ---
            def load_T(sb, dr, n, eng, parts=2):
                nk = dr.shape[0] // 128
                for i in range(parts):
                    a, b = i * nk // parts, (i + 1) * nk // parts
                    eng.dma_start(
                        sb[:, a * n:b * n].rearrange("p (k f) -> p k f", f=n),
                        dr[a * 128:b * 128, :].rearrange("(k p) f -> p k f", p=128))
            nc.sync.dma_start(xT[:], d_x0[:])
            nc.sync.dma_start(c0[:], d_c0[:])
            nc.sync.dma_start(c1[:], d_c1[:])
            load_T(h0T, d_h0, B, nc.scalar, 1)
            load_T(h1T, d_h1, B, nc.sync, 1)
            nc.scalar.dma_start(wih0[:], d_wih0[:])
            load_T(whh0, d_whh0, G, nc.scalar)
            load_T(whh1, d_whh1, G, nc.sync)
            load_T(wih1, d_wih1, G, nc.scalar)
            load_T(fcw, d_fcw, OUT, nc.sync, 1)
            nc.sync.dma_start(b1[:], d_b1[:])
            nc.sync.dma_start(fcb[:], d_fcb[:])
            nc.sync.dma_start(ones[:], d_ones[:])
            nc.sync.dma_start(id64[:], d_id[:])

            def fc_block(t):
                """out[t] = clip(fc(h1(t))); writes out DRAM row t and xT."""
                ps = psF.tile([OUT, B], f32, tag="fc")
                for k in range(NK):
                    nc.tensor.matmul(
                        ps[:], fcw[:, k * OUT:(k + 1) * OUT], h1T[:, k * B:(k + 1) * B],
                        start=(k == 0), stop=False)
                nc.tensor.matmul(ps[:], fcb[0:1, :], ones[0:1, :], start=False, stop=True)
                o_sb = work.tile([OUT, B], f32, tag="osb")
                nc.vector.tensor_scalar(o_sb[:], ps[:], 1.0, -1.0, op0=ALU.min, op1=ALU.max)
                nc.sync.dma_start(d_out[t], o_sb[:])
                nc.vector.tensor_copy(xT[0:IN, :], o_sb[:])  # f32 -> bf16 cast

            def gates_nonlin(layer, ps, c, t):
                """PSUM gates [128, 512] -> h sliceT bf16 in DRAM bounce.

                gate row order is [i, f, o, g] (host reorders); col-group A
                wrote gate cols 0:512 (i, f) on partitions 0:64, col-group B
                wrote cols 512:1024 (o, g) on partitions 64:128."""
                act = work.tile([B, G], f32, tag="gact")
                nc.scalar.activation(act[:, 0:512], ps[0:B, :], AF.Sigmoid)              # i, f
                nc.scalar.activation(act[:, 3 * HS:4 * HS], ps[B:2 * B, HS:2 * HS], AF.Tanh)  # g
                nc.scalar.activation(act[:, 512:3 * HS], ps[B:2 * B, 0:HS], AF.Sigmoid)     # o
                # state update pipelined in two hidden-unit halves so the
                # transpose/cast/DMA of half 0 overlaps the math of half 1
                ag_in = dram.tile([HS, B], bf16, tag=f"agin{layer}")
                pst = psTpool.tile([128, 128], f32, tag="tr")
                hTbf = work.tile([128, 2 * B], bf16, tag="hTbf")
                t1 = work.tile([B, HS], f32, tag="t1")
                t2 = work.tile([B, HS], f32, tag="t2")
                tc_ = work.tile([B, HS], f32, tag="tch")
                hs = work.tile([B, HS], f32, tag="hs")
                for u in (0, 1):
                    s = slice(u * 128, (u + 1) * 128)
                    nc.vector.tensor_mul(t2[:, s], act[:, u * 128:u * 128 + 128],
                                         act[:, 3 * HS + u * 128:3 * HS + u * 128 + 128])  # i*g
                    nc.vector.tensor_mul(t1[:, s], act[:, HS + u * 128:HS + u * 128 + 128], c[:, s])  # f*c
                    nc.vector.tensor_add(c[:, s], t1[:, s], t2[:, s])                # c_new
                    nc.scalar.activation(tc_[:, s], c[:, s], AF.Tanh)
                    nc.vector.tensor_mul(hs[:, s], act[:, 2 * HS + u * 128:2 * HS + u * 128 + 128],
                                         tc_[:, s])                                  # o * tanh(c)
                    nc.tensor.transpose(pst[:, u * B:(u + 1) * B], hs[:, s], id64[:])
                    nc.vector.tensor_copy(hTbf[:, u * B:(u + 1) * B], pst[:, u * B:(u + 1) * B])
                    eng = nc.scalar if u == 0 else nc.sync
                    eng.dma_start(ag_in[u * 128:(u + 1) * 128, :], hTbf[:, u * B:(u + 1) * B])
                return ag_in

            def all_gather(layer, ag_in, hT):
                ag_out = dram.tile([H, B], bf16, tag=f"agout{layer}", addr_space="Shared")
                nc.gpsimd.collective_compute(
                    "AllGather", ALU.bypass, replica_groups=groups,
                    ins=[ag_in[:].opt()], outs=[ag_out[:].opt()])
                # staggered quarter readbacks on both HWDGE sequencers so the
                # first consuming matmuls start earlier
                for q, eng in ((0, nc.scalar), (1, nc.sync), (2, nc.scalar), (3, nc.sync)):
                    eng.dma_start(
                        hT[:, q * 4 * B:(q + 1) * 4 * B].rearrange("p (k f) -> p k f", f=B),
                        ag_out[q * 512:(q + 1) * 512, :].rearrange("(k p) f -> p k f", p=128))

            def mm_group(ps, hT_src, w, start, stop):
                """Col-tiled matmul pairs: group A (PE cols 0:63) computes
                gate cols 0:512 -> partitions 0:64; group B (cols 64:127)
                computes gate cols 512:1024 -> partitions 64:128. The two
                stream concurrently through the PE array."""
                for k in range(NK):
                    for r in range(2):
                        nc.tensor.matmul(
                            ps[r * B:(r + 1) * B, :],
                            hT_src[:, k * B:(k + 1) * B],
                            w[:, k * G + r * 512: k * G + (r + 1) * 512],
                            start=(start and k == 0),
                            stop=(stop and k == NK - 1),
                            tile_position=(0, r * B))

            def xb_pair(ps, lhsT, rhs2, start, stop):
                """One K-chunk into both col groups (x-part / bias)."""
                for r in range(2):
                    nc.tensor.matmul(
                        ps[r * B:(r + 1) * B, :],
                        lhsT, rhs2[0:lhsT.shape[0], r * 512:(r + 1) * 512],
                        start=start, stop=stop, tile_position=(0, r * B))

            for t in range(T):
                # ---- layer 0 gates: h-part first (ready before FC of t-1) ----
                g0 = psA.tile([2 * B, 512], f32, tag="g")
                mm_group(g0, h0T, whh0, start=True, stop=False)
                # ---- FC of previous step (needs h1T refreshed by AG1(t-1)) ----
                if t > 0:
                    fc_block(t - 1)
                # ---- layer 0 x-part (closes both accumulation groups) ----
                xb_pair(g0, xT[:], wih0[:], start=False, stop=True)
                ag0_in = gates_nonlin(0, g0, c0, t)
                # ---- layer 1 h-part (independent of AG0 -> overlaps collective) ----
                g1 = psA.tile([2 * B, 512], f32, tag="g")
                mm_group(g1, h1T, whh1, start=True, stop=False)
                xb_pair(g1, ones[0:1, :], b1[:], start=False, stop=False)
                all_gather(0, ag0_in, h0T)
                # ---- layer 1 x-part: W_ih1 @ h0(t) ----
                mm_group(g1, h0T, wih1, start=False, stop=True)
                ag1_in = gates_nonlin(1, g1, c1, t)
                all_gather(1, ag1_in, h1T)
            fc_block(T - 1)

    nc.compile()
    return nc


def _prep_inputs(inputs, hiddens, cells, W_ih0, W_hh0, b_ih0, b_hh0,
                 W_ih1, W_hh1, b_ih1, b_hh1, fc_w, fc_b):
    """Host-side sharding: per-core in_maps."""
    b0 = np.asarray(b_ih0, np.float32) + np.asarray(b_hh0, np.float32)
    b1 = np.asarray(b_ih1, np.float32) + np.asarray(b_hh1, np.float32)
    onesB = np.ones((1, B), np.float32)
    x0T = np.concatenate([np.asarray(inputs, np.float32).T, onesB], 0)
    shared = {
        "fcw": np.ascontiguousarray(np.asarray(fc_w, np.float32).T).astype(BF),
        "fcb": np.asarray(fc_b, np.float32)[None, :].astype(BF),
        "x0": x0T.astype(BF),
        "h0": np.ascontiguousarray(np.asarray(hiddens[0], np.float32).T).astype(BF),
        "h1": np.ascontiguousarray(np.asarray(hiddens[1], np.float32).T).astype(BF),
        "ones": onesB.astype(BF),
        "id64": np.eye(B, dtype=np.float32),
    }
    in_maps = []
    for d in range(R):
        def rows(W):
            # gate order [i, f, o, g] (torch order is i, f, g, o) — kernel's
            # nonlin slices assume the three sigmoids are contiguous
            W = np.asarray(W, np.float32)
            return np.concatenate([W[g * H + d * HS: g * H + (d + 1) * HS] for g in (0, 1, 3, 2)], 0)
        wih0T = rows(W_ih0).T                      # [66, 1024]
        bias0 = rows(b0[:, None])[:, 0][None, :]   # [1, 1024]
        m = dict(shared)
        m["wih0"] = np.concatenate([wih0T, bias0], 0).astype(BF)
        m["whh0"] = np.ascontiguousarray(rows(W_hh0).T).astype(BF)
        m["wih1"] = np.ascontiguousarray(rows(W_ih1).T).astype(BF)
        m["whh1"] = np.ascontiguousarray(rows(W_hh1).T).astype(BF)
        m["b1"] = rows(b1[:, None])[:, 0][None, :].astype(BF)
        m["c0"] = np.ascontiguousarray(np.asarray(cells[0], np.float32)[:, d * HS:(d + 1) * HS])
        m["c1"] = np.ascontiguousarray(np.asarray(cells[1], np.float32)[:, d * HS:(d + 1) * HS])
        in_maps.append(m)
    return in_maps


def _get_nc():
    if "nc" not in _CACHE:
        _CACHE["nc"] = _build()
    return _CACHE["nc"]


def run_raw(in_maps, trace=False, **kw):
    from concourse import bass_utils
    nc = _get_nc()
    return bass_utils.run_bass_kernel_spmd(
        nc, in_maps, core_ids=list(range(R)), trace=trace, **kw)


def kernel(**inputs):
    in_maps = _prep_inputs(**inputs)
    res = run_raw(in_maps)
    outT = np.asarray(res.results[0]["out"])          # [25, 66, 64]
    return np.ascontiguousarray(outT.transpose(2, 0, 1)).astype(np.float32)
